# revision 1
# baseline (speedup 1.0000x reference)
"""Trainium2 Bass kernel for nn_Jointer: per-sample masked cosine-similarity.

out[b] = relu(l2norm(source[b]) @ l2norm(target[b]).T) * (mask_src[b] outer mask_tar[b])

Sharding: data-parallel over batch B=8 -> one sample per NeuronCore.
Per core: normalize+mask fold, PE-transpose both operands to [D, tokens],
fp32r matmul in 128x512 tiles, fused scale+relu out of PSUM, 1MB row DMAs.
"""

import numpy as np

import concourse.bass as bass
from concourse import bacc
import concourse.mybir as mybir
import concourse.tile as tile
from concourse.bass_utils import run_bass_kernel_spmd
from concourse.masks import make_identity

F32 = mybir.dt.float32
F32R = mybir.dt.float32r
AF = mybir.ActivationFunctionType
ALU = mybir.AluOpType

S = 2048  # source tokens per sample
T = 2048  # target tokens per sample
D = 128  # feature dim (= contraction dim = partitions)
P = 128  # partitions
SB = S // P  # 16 source token blocks
TB = T // P  # 16 target token blocks
NT = 512  # matmul moving free dim (one PSUM bank of fp32)
NCHUNKS = T // NT  # 4


def build_nc() -> bass.Bass:
    nc = bacc.Bacc(trn_type="TRN2")

    src = nc.dram_tensor("src", [S, D], F32, kind="ExternalInput")
    tgt = nc.dram_tensor("tgt", [T, D], F32, kind="ExternalInput")
    # maskf[p, k]: k in [0,16) source-block masks, k in [16,32) target-block
    # masks; value for token 128*k + p.
    maskf = nc.dram_tensor("maskf", [P, SB + TB], F32, kind="ExternalInput")
    out = nc.dram_tensor("out", [S, T], F32, kind="ExternalOutput")

    src_r = src.rearrange("(k p) d -> p k d", p=P)
    tgt_r = tgt.rearrange("(k p) d -> p k d", p=P)
    out_r = out.rearrange("(m p) n -> m p n", p=P)
    mask_r = maskf.rearrange("p k -> p k")

    G = 4  # blocks per pipeline group
    NG = TB // G  # 4 groups

    with tile.TileContext(nc) as tc:
        with (
            tc.tile_pool(name="singles", bufs=1) as singles,
            tc.tile_pool(name="inbuf", bufs=1) as inbuf,
            tc.tile_pool(name="sq", bufs=2) as sqpool,
            tc.tile_pool(name="norm", bufs=1) as normp,
            tc.tile_pool(name="tscl", bufs=3) as tsclp,
            tc.tile_pool(name="pst", bufs=2, space="PSUM") as psum_t,
            tc.tile_pool(name="psmm", bufs=4, space="PSUM") as psum_mm,
            tc.tile_pool(name="outp", bufs=4) as outp,
        ):
            ident = singles.tile([P, P], F32)
            make_identity(nc, ident)

            mask_sb = singles.tile([P, SB + TB], F32)
            nc.sync.dma_start(out=mask_sb, in_=mask_r)

            s_nat = inbuf.tile([P, SB, D], F32)
            sT = inbuf.tile([P, S], F32R)  # [D, s tokens] (raw)
            s_scl = normp.tile([P, SB], F32)
            t_nat = inbuf.tile([P, TB, D], F32)
            tT = inbuf.tile([P, T], F32R)  # [D, t tokens] normalized+masked
            t_scl = normp.tile([P, TB], F32)

            def s_load(g):
                blk = slice(g * G, (g + 1) * G)
                nc.sync.dma_start(out=s_nat[:, blk, :], in_=src_r[:, blk, :])
                ps = psum_t.tile([P, G * P], F32, tag="pst", name=f"ps_s{g}")
                for j in range(G):
                    k = g * G + j
                    nc.tensor.transpose(
                        ps[:, j * P : (j + 1) * P], s_nat[:, k, :], ident
                    )
                nc.vector.tensor_copy(
                    out=sT[:, g * G * P : (g + 1) * G * P], in_=ps
                )

            def s_norm(g):
                blk = slice(g * G, (g + 1) * G)
                s_sq = sqpool.tile([P, G, D], F32, tag="sq", name=f"ssq{g}")
                nc.scalar.activation(out=s_sq, in_=s_nat[:, blk, :], func=AF.Square)
                s_ss = normp.tile([P, G], F32, tag="sss", name=f"sss{g}")
                nc.vector.reduce_sum(out=s_ss, in_=s_sq, axis=mybir.AxisListType.X)
                s_rcp = normp.tile([P, G], F32, tag="srcp", name=f"srcp{g}")
                nc.vector.reciprocal(out=s_rcp, in_=s_ss)
                s_rsq = normp.tile([P, G], F32, tag="srsq", name=f"srsq{g}")
                nc.scalar.activation(out=s_rsq, in_=s_rcp, func=AF.Sqrt)
                nc.vector.tensor_mul(
                    out=s_scl[:, blk],
                    in0=s_rsq,
                    in1=mask_sb[:, g * G : (g + 1) * G],
                )

            t_rsqs = {}

            def t_norm(g):
                blk = slice(g * G, (g + 1) * G)
                nc.sync.dma_start(out=t_nat[:, blk, :], in_=tgt_r[:, blk, :])
                t_sq = sqpool.tile([P, G, D], F32, tag="sq", name=f"tsq{g}")
                nc.scalar.activation(out=t_sq, in_=t_nat[:, blk, :], func=AF.Square)
                t_ss = normp.tile([P, G], F32, tag="tss", name=f"tss{g}")
                nc.vector.reduce_sum(out=t_ss, in_=t_sq, axis=mybir.AxisListType.X)
                t_rcp = normp.tile([P, G], F32, tag="trcp", name=f"trcp{g}")
                nc.vector.reciprocal(out=t_rcp, in_=t_ss)
                t_rsq = normp.tile([P, G], F32, tag="trsq", name=f"trsq{g}")
                nc.scalar.activation(out=t_rsq, in_=t_rcp, func=AF.Sqrt)
                t_rsqs[g] = t_rsq

            def t_xpose(g):
                # scale*mask + transpose 4 blocks; two half-bank copies run on
                # ACT and DVE in parallel to cut the chain latency.
                t_rsq = t_rsqs[g]
                ps = psum_t.tile([P, G * P], F32, tag="pst", name=f"ps_t{g}")
                for j in range(G):
                    k = g * G + j
                    t_sc = tsclp.tile([P, P], F32, tag="tscl")
                    nc.vector.tensor_scalar(
                        out=t_sc,
                        in0=t_nat[:, k, :],
                        scalar1=t_rsq[:, j : j + 1],
                        scalar2=mask_sb[:, SB + k : SB + k + 1],
                        op0=ALU.mult,
                        op1=ALU.mult,
                    )
                    nc.tensor.transpose(ps[:, j * P : (j + 1) * P], t_sc, ident)
                half = G * P // 2
                base = g * G * P
                nc.scalar.copy(out=tT[:, base : base + half], in_=ps[:, 0:half])
                nc.vector.tensor_copy(
                    out=tT[:, base + half : base + 2 * half],
                    in_=ps[:, half : 2 * half],
                )

            # --- main matmul + fused (scale * relu) + output DMA.
            # First rows stream per-512-chunk DMAs so the DMA queue saturates
            # as soon as the first tT chunk lands; later rows use 1MB row DMAs.
            EARLY_ROWS = 2
            ob_tiles = {}

            def mm_chunk(m, n):
                if m not in ob_tiles:
                    ob_tiles[m] = outp.tile([P, T], F32, tag="ob", name=f"ob{m}")
                ob = ob_tiles[m]
                ps = psum_mm.tile([P, NT], F32, tag="psmm", name=f"mm{m}_{n}")
                nc.tensor.matmul(
                    ps,
                    sT[:, m * P : (m + 1) * P],
                    tT[:, n * NT : (n + 1) * NT],
                    start=True,
                    stop=True,
                )
                dst = ob[:, n * NT : (n + 1) * NT]
                if (m * NCHUNKS + n) % 2 == 0:
                    nc.scalar.activation(
                        out=dst, in_=ps, func=AF.Relu, scale=s_scl[:, m : m + 1]
                    )
                else:
                    nc.vector.tensor_scalar(
                        out=dst,
                        in0=ps,
                        scalar1=s_scl[:, m : m + 1],
                        scalar2=0.0,
                        op0=ALU.mult,
                        op1=ALU.max,
                    )
                if m < EARLY_ROWS:
                    nc.sync.dma_start(
                        out=out_r[m][:, n * NT : (n + 1) * NT], in_=dst
                    )
                elif n == NCHUNKS - 1:
                    nc.sync.dma_start(out=out_r[m], in_=ob)

            def mm_row(m):
                for n in range(NCHUNKS):
                    mm_chunk(m, n)

            # Emission order == per-engine FIFO order, so it must match data
            # readiness: t0's norm chain leads the ACT/DVE FIFOs (it is the
            # critical path to the first output chunk), s0's transposes lead
            # the PE FIFO (their data lands first), and row-0 chunks
            # interleave with the t groups that feed them.  Remaining s
            # groups fill engine gaps between row batches.
            t_norm(0)
            s_load(0)
            t_xpose(0)
            s_norm(0)
            mm_chunk(0, 0)
            t_norm(1)
            t_xpose(1)
            mm_chunk(0, 1)
            t_norm(2)
            t_xpose(2)
            mm_chunk(0, 2)
            t_norm(3)
            t_xpose(3)
            mm_chunk(0, 3)
            mm_row(1)
            s_load(1)
            mm_row(2)
            s_norm(1)
            mm_row(3)
            s_load(2)
            mm_row(4)
            s_norm(2)
            mm_row(5)
            mm_row(6)
            s_load(3)
            mm_row(7)
            s_norm(3)
            for m in range(8, 16):
                mm_row(m)

    nc.compile()
    return nc


_NC_CACHE = None


def _get_nc():
    global _NC_CACHE
    if _NC_CACHE is None:
        _NC_CACHE = build_nc()
    return _NC_CACHE


def kernel(source, target, mask_src, mask_tar, **run_kwargs):
    source = np.asarray(source, dtype=np.float32)
    target = np.asarray(target, dtype=np.float32)
    mask_src = np.asarray(mask_src)
    mask_tar = np.asarray(mask_tar)
    B = source.shape[0]

    in_maps = []
    for b in range(B):
        msf = mask_src[b].astype(np.float32).reshape(SB, P).T
        mtf = mask_tar[b].astype(np.float32).reshape(TB, P).T
        mk = np.ascontiguousarray(np.concatenate([msf, mtf], axis=1))
        in_maps.append(
            {
                "src": np.ascontiguousarray(source[b]),
                "tgt": np.ascontiguousarray(target[b]),
                "maskf": mk,
            }
        )

    nc = _get_nc()
    res = run_bass_kernel_spmd(nc, in_maps, core_ids=list(range(B)), **run_kwargs)
    out = np.stack([r["out"] for r in res.results], axis=0)
    if run_kwargs.get("trace"):
        kernel.last_results = res
    return out



# revision 3
# speedup vs baseline: 1.3136x; 1.3136x over previous
"""Trainium2 Bass kernel for nn_Jointer: per-sample masked cosine-similarity.

out[b] = relu(l2norm(source[b]) @ l2norm(target[b]).T) * (mask_src[b] outer mask_tar[b])

Sharding: data-parallel over batch B=8 -> one sample per NeuronCore.

Strategy (memory-bound problem; rel-err budget 2e-2 permits bf16 I/O):
- Host casts source/target to bf16 (halves input DMA bytes); kernel writes a
  bf16 output that the host upcasts to f32 (halves the dominant 16 MB output
  stream). Norm statistics and matmul accumulation stay fp32; measured end-to-
  end rel err ~4e-3.
- t operands are scaled by rsqrt(ss)*mask BEFORE the PE transpose; s operands
  are transposed RAW and their rsqrt(ss)*mask scale is fused into the
  PSUM->SBUF relu pass (per-row scalar), so the s-transposes have no
  dependency on the ACT table load / sqrt chain.
- All input DMAs are issued up-front on the sync queue in dependency order;
  output starts streaming as a column-major "band" over rows 0-3 (each band
  segment only needs the t-blocks transposed so far), then rows 4-15 go
  row-major with 1 MB row-pair DMAs. This keeps the single HWDGE queue busy
  from the first input byte to the last output byte.
- PSUM->SBUF relu ops work on [128,1024] two-bank tiles to amortize the ACT
  engine's 352-cycle fixed overhead; ops alternate ACT/DVE.
"""

import numpy as np
import ml_dtypes

import concourse.bass as bass
from concourse import bacc
import concourse.mybir as mybir
import concourse.tile as tile
from concourse.bass_utils import run_bass_kernel_spmd
from concourse.masks import make_identity

F32 = mybir.dt.float32
BF16 = mybir.dt.bfloat16
AF = mybir.ActivationFunctionType
ALU = mybir.AluOpType
AX = mybir.AxisListType

S = 2048  # source tokens per sample
T = 2048  # target tokens per sample
D = 128  # feature dim (= contraction dim = partitions)
P = 128  # partitions
SB = S // P  # 16 source token blocks
TB = T // P  # 16 target token blocks

BF = ml_dtypes.bfloat16


def build_nc() -> bass.Bass:
    nc = bacc.Bacc(trn_type="TRN2")

    src = nc.dram_tensor("src", [S, D], BF16, kind="ExternalInput")
    tgt = nc.dram_tensor("tgt", [T, D], BF16, kind="ExternalInput")
    # maskf[p, k]: k in [0,16) source-block masks, k in [16,32) target-block
    # masks; value for token 128*k + p.
    maskf = nc.dram_tensor("maskf", [P, SB + TB], F32, kind="ExternalInput")
    out = nc.dram_tensor("out", [S, T], BF16, kind="ExternalOutput")

    src_r = src.rearrange("(k p) d -> p k d", p=P)
    tgt_r = tgt.rearrange("(k p) d -> p k d", p=P)
    out_pm = out.rearrange("(m p) n -> p m n", p=P)  # [P, 16, 2048]

    with tile.TileContext(nc) as tc:
        with (
            tc.tile_pool(name="singles", bufs=1) as singles,
            tc.tile_pool(name="inbuf", bufs=1) as inbuf,
            tc.tile_pool(name="sq", bufs=2) as sqp,
            tc.tile_pool(name="scl", bufs=4) as sclp,
            tc.tile_pool(name="pst", bufs=2, space="PSUM") as pst,
            tc.tile_pool(name="psmm", bufs=3, space="PSUM") as psmm,
            tc.tile_pool(name="bandp", bufs=1) as bandp,
            tc.tile_pool(name="outp", bufs=3) as outp,
        ):
            ident = singles.tile([P, P], BF16)
            make_identity(nc, ident)

            mask_sb = singles.tile([P, SB + TB], F32)
            nc.sync.dma_start(out=mask_sb, in_=maskf.rearrange("p k -> p k"))

            s_nat = inbuf.tile([P, SB, D], BF16)
            t_nat = inbuf.tile([P, TB, D], BF16)
            sT = inbuf.tile([P, S], BF16)  # [D, s tokens] raw (scale in relu)
            tT = inbuf.tile([P, T], BF16)  # [D, t tokens] normalized+masked

            ss_t = singles.tile([P, TB], F32)
            rc_t = singles.tile([P, TB], F32)
            rq_t = singles.tile([P, TB], F32)
            ss_s = singles.tile([P, SB], F32)
            rc_s = singles.tile([P, SB], F32)
            rq_s = singles.tile([P, SB], F32)
            s_scl = singles.tile([P, SB], F32)  # rsqrt * mask, per s block

            # ---- input DMAs: issued up-front, FIFO order == drain order.
            nc.sync.dma_start(out=t_nat[:, 0:2, :], in_=tgt_r[:, 0:2, :])
            nc.sync.dma_start(out=s_nat[:, 0:4, :], in_=src_r[:, 0:4, :])
            nc.sync.dma_start(out=t_nat[:, 2:4, :], in_=tgt_r[:, 2:4, :])
            nc.sync.dma_start(out=t_nat[:, 4:8, :], in_=tgt_r[:, 4:8, :])
            nc.sync.dma_start(out=t_nat[:, 8:16, :], in_=tgt_r[:, 8:16, :])
            nc.sync.dma_start(out=s_nat[:, 4:16, :], in_=src_r[:, 4:16, :])

            def norm(x_nat, ss, rc, rq, lo, n, tag):
                """sum-of-squares -> 1/x -> sqrt => rq = 1/||x|| per token."""
                sq = sqp.tile([P, n, D], BF16, tag="sq", name=f"sq_{tag}{lo}")
                nc.vector.tensor_mul(
                    out=sq, in0=x_nat[:, lo : lo + n, :], in1=x_nat[:, lo : lo + n, :]
                )
                nc.vector.reduce_sum(out=ss[:, lo : lo + n], in_=sq, axis=AX.X)
                nc.vector.reciprocal(out=rc[:, lo : lo + n], in_=ss[:, lo : lo + n])
                nc.scalar.activation(
                    out=rq[:, lo : lo + n], in_=rc[:, lo : lo + n], func=AF.Sqrt
                )

            def xpose_t(lo, n):
                """scale+mask t blocks, PE-transpose, copy to tT (ACT|DVE halves)."""
                ps = pst.tile([P, 512], BF16, tag="pst", name=f"xpt{lo}")
                for j in range(n):
                    k = lo + j
                    xs = sclp.tile([P, P], BF16, tag="scl")
                    nc.vector.tensor_scalar(
                        out=xs,
                        in0=t_nat[:, k, :],
                        scalar1=rq_t[:, k : k + 1],
                        scalar2=mask_sb[:, SB + k : SB + k + 1],
                        op0=ALU.mult,
                        op1=ALU.mult,
                    )
                    nc.tensor.transpose(ps[:, j * P : (j + 1) * P], xs, ident)
                half = n * P // 2
                base = lo * P
                nc.scalar.copy(out=tT[:, base : base + half], in_=ps[:, 0:half])
                nc.vector.tensor_copy(
                    out=tT[:, base + half : base + n * P], in_=ps[:, half : n * P]
                )

            def xpose_s(lo, n):
                """PE-transpose raw s blocks (no scale dependency)."""
                ps = pst.tile([P, 512], BF16, tag="pst", name=f"xps{lo}")
                for j in range(n):
                    k = lo + j
                    nc.tensor.transpose(ps[:, j * P : (j + 1) * P], s_nat[:, k, :], ident)
                half = n * P // 2
                base = lo * P
                nc.scalar.copy(out=sT[:, base : base + half], in_=ps[:, 0:half])
                nc.vector.tensor_copy(
                    out=sT[:, base + half : base + n * P], in_=ps[:, half : n * P]
                )

            alt = [0]

            def out_op(dst, ps_ap, m):
                """relu(scale * psum) -> bf16 SBUF, alternating ACT/DVE."""
                if alt[0] % 2 == 0:
                    nc.scalar.activation(
                        out=dst, in_=ps_ap, func=AF.Relu, scale=s_scl[:, m : m + 1]
                    )
                else:
                    nc.vector.tensor_scalar(
                        out=dst,
                        in0=ps_ap,
                        scalar1=s_scl[:, m : m + 1],
                        scalar2=0.0,
                        op0=ALU.mult,
                        op1=ALU.max,
                    )
                alt[0] += 1

            # band output tiles for rows 0-3: [P, m-pair, T]
            ob01 = bandp.tile([P, 2, T], BF16, name="ob01")
            ob23 = bandp.tile([P, 2, T], BF16, name="ob23")
            band_obs = [(ob01, 0), (ob23, 2)]

            def band_seg(c0, w):
                """rows 0-3, columns [c0, c0+w) with w <= 512."""
                for ob, mlo in band_obs:
                    ps = psmm.tile([P, 1024], F32, tag="mm", name=f"b{c0}_{mlo}")
                    for i in range(2):
                        m = mlo + i
                        nc.tensor.matmul(
                            ps[:, i * w : (i + 1) * w],
                            sT[:, m * P : (m + 1) * P],
                            tT[:, c0 : c0 + w],
                            start=True,
                            stop=True,
                        )
                    for i in range(2):
                        m = mlo + i
                        out_op(
                            ob[:, i, c0 : c0 + w], ps[:, i * w : (i + 1) * w], m
                        )
                    nc.sync.dma_start(
                        out=out_pm[:, mlo : mlo + 2, c0 : c0 + w],
                        in_=ob[:, :, c0 : c0 + w],
                    )

            def band_segD():
                """rows 0-3, columns [1024, 2048)."""
                for ob, mlo in band_obs:
                    for i in range(2):
                        m = mlo + i
                        ps = psmm.tile([P, 1024], F32, tag="mm", name=f"bD_{m}")
                        nc.tensor.matmul(
                            ps[:, 0:512],
                            sT[:, m * P : (m + 1) * P],
                            tT[:, 1024:1536],
                            start=True,
                            stop=True,
                        )
                        nc.tensor.matmul(
                            ps[:, 512:1024],
                            sT[:, m * P : (m + 1) * P],
                            tT[:, 1536:2048],
                            start=True,
                            stop=True,
                        )
                        out_op(ob[:, i, 1024:2048], ps, m)
                    nc.sync.dma_start(
                        out=out_pm[:, mlo : mlo + 2, 1024:2048],
                        in_=ob[:, :, 1024:2048],
                    )

            def row_pair(mlo):
                """rows mlo, mlo+1 row-major; one 1 MB pair DMA."""
                obp = outp.tile([P, 2, T], BF16, tag="obp", name=f"obp{mlo}")
                for i in range(2):
                    m = mlo + i
                    for h in range(2):
                        c = h * 1024
                        ps = psmm.tile([P, 1024], F32, tag="mm", name=f"r{m}_{h}")
                        nc.tensor.matmul(
                            ps[:, 0:512],
                            sT[:, m * P : (m + 1) * P],
                            tT[:, c : c + 512],
                            start=True,
                            stop=True,
                        )
                        nc.tensor.matmul(
                            ps[:, 512:1024],
                            sT[:, m * P : (m + 1) * P],
                            tT[:, c + 512 : c + 1024],
                            start=True,
                            stop=True,
                        )
                        out_op(obp[:, i, c : c + 1024], ps, m)
                nc.sync.dma_start(out=out_pm[:, mlo : mlo + 2, :], in_=obp)

            # ---- emission order == per-engine FIFO order.
            # DVE: t01 chain leads (critical path to first tT columns); the s
            # scale chain computes s_scl (needed by the first out_op, not by
            # the s transposes). PE: raw s transposes lead (data lands first,
            # no scale dependency), then t transposes and band matmuls
            # interleave with t-group readiness.
            norm(t_nat, ss_t, rc_t, rq_t, 0, 2, "t")  # t01
            norm(t_nat, ss_t, rc_t, rq_t, 2, 2, "t")  # t23
            norm(s_nat, ss_s, rc_s, rq_s, 0, 4, "s")  # s03 (for s_scl)
            nc.vector.tensor_mul(
                out=s_scl[:, 0:4], in0=rq_s[:, 0:4], in1=mask_sb[:, 0:4]
            )
            xpose_s(0, 4)
            xpose_t(0, 2)
            xpose_t(2, 2)
            norm(t_nat, ss_t, rc_t, rq_t, 4, 4, "t")  # t47
            band_seg(0, 256)
            band_seg(256, 256)
            xpose_t(4, 4)
            norm(t_nat, ss_t, rc_t, rq_t, 8, 4, "t")
            band_seg(512, 512)
            xpose_t(8, 4)
            norm(t_nat, ss_t, rc_t, rq_t, 12, 4, "t")
            xpose_t(12, 4)
            norm(s_nat, ss_s, rc_s, rq_s, 4, 4, "s")
            nc.vector.tensor_mul(
                out=s_scl[:, 4:8], in0=rq_s[:, 4:8], in1=mask_sb[:, 4:8]
            )
            band_segD()
            xpose_s(4, 4)
            row_pair(4)
            norm(s_nat, ss_s, rc_s, rq_s, 8, 4, "s")
            nc.vector.tensor_mul(
                out=s_scl[:, 8:12], in0=rq_s[:, 8:12], in1=mask_sb[:, 8:12]
            )
            row_pair(6)
            xpose_s(8, 4)
            row_pair(8)
            norm(s_nat, ss_s, rc_s, rq_s, 12, 4, "s")
            nc.vector.tensor_mul(
                out=s_scl[:, 12:16], in0=rq_s[:, 12:16], in1=mask_sb[:, 12:16]
            )
            row_pair(10)
            xpose_s(12, 4)
            row_pair(12)
            row_pair(14)

    nc.compile()
    return nc


_NC_CACHE = None


def _get_nc():
    global _NC_CACHE
    if _NC_CACHE is None:
        _NC_CACHE = build_nc()
    return _NC_CACHE


def kernel(source, target, mask_src, mask_tar, **run_kwargs):
    source = np.asarray(source, dtype=np.float32)
    target = np.asarray(target, dtype=np.float32)
    mask_src = np.asarray(mask_src)
    mask_tar = np.asarray(mask_tar)
    B = source.shape[0]

    in_maps = []
    for b in range(B):
        msf = mask_src[b].astype(np.float32).reshape(SB, P).T
        mtf = mask_tar[b].astype(np.float32).reshape(TB, P).T
        mk = np.ascontiguousarray(np.concatenate([msf, mtf], axis=1))
        in_maps.append(
            {
                "src": np.ascontiguousarray(source[b].astype(BF)),
                "tgt": np.ascontiguousarray(target[b].astype(BF)),
                "maskf": mk,
            }
        )

    nc = _get_nc()
    res = run_bass_kernel_spmd(nc, in_maps, core_ids=list(range(B)), **run_kwargs)
    out = np.stack(
        [np.asarray(r["out"]).astype(np.float32) for r in res.results], axis=0
    )
    if run_kwargs.get("trace"):
        kernel.last_results = res
    return out


# revision 5
# speedup vs baseline: 1.3139x; 1.0002x over previous
"""Trainium2 Bass kernel for nn_Jointer: per-sample masked cosine-similarity.

out[b] = relu(l2norm(source[b]) @ l2norm(target[b]).T) * (mask_src[b] outer mask_tar[b])

Sharding: data-parallel over batch B=8 -> one sample per NeuronCore.

Strategy (memory-bound problem; rel-err budget 2e-2 permits bf16 I/O):
- Host casts source/target to bf16 (halves input DMA bytes); kernel writes a
  bf16 output that the host upcasts to f32 (halves the dominant 16 MB output
  stream). Norm statistics and matmul accumulation stay fp32; measured end-to-
  end rel err ~3.3e-3.
- Engine roles: GpSimd does the SBUF-side elementwise prep (squares, reduces,
  scale*mask) so the DVE/ACT FIFOs stay clear for the PSUM-drain relu ops;
  DVE does reciprocals + half the relu ops + half the transpose copies; ACT
  does sqrt + the other halves. PE does transposes + matmuls only.
- t operands are scaled by rsqrt(ss)*mask BEFORE the PE transpose; s operands
  are transposed RAW and their rsqrt(ss)*mask scale is fused into the
  PSUM->SBUF relu pass (per-row scalar), so the s-transposes depend only on
  the s input DMA.
- Inputs load via two queues in parallel (sync HWDGE + gpsimd SWDGE); outputs
  stream on the sync queue as a column-major "band" over rows 0-3 (each band
  segment needs only the t-blocks transposed so far), then rows 4-15 go
  row-major with 1 MB row-pair DMAs.
- PSUM->SBUF relu ops work on [128,1024] two-bank tiles to amortize the ACT
  engine's fixed overhead; ops alternate ACT/DVE. Transpose PSUM tiles are
  padded to a full bank so PE writes never share a bank with ACT/DVE reads.
"""

import numpy as np
import ml_dtypes

import concourse.bass as bass
from concourse import bacc
import concourse.mybir as mybir
import concourse.tile as tile
from concourse.bass_utils import run_bass_kernel_spmd
from concourse.masks import make_identity

F32 = mybir.dt.float32
BF16 = mybir.dt.bfloat16
AF = mybir.ActivationFunctionType
ALU = mybir.AluOpType
AX = mybir.AxisListType

S = 2048  # source tokens per sample
T = 2048  # target tokens per sample
D = 128  # feature dim (= contraction dim = partitions)
P = 128  # partitions
SB = S // P  # 16 source token blocks
TB = T // P  # 16 target token blocks

BF = ml_dtypes.bfloat16


def build_nc() -> bass.Bass:
    nc = bacc.Bacc(trn_type="TRN2")

    src = nc.dram_tensor("src", [S, D], BF16, kind="ExternalInput")
    tgt = nc.dram_tensor("tgt", [T, D], BF16, kind="ExternalInput")
    # maskf[p, k]: k in [0,16) source-block masks, k in [16,32) target-block
    # masks; value for token 128*k + p.
    maskf = nc.dram_tensor("maskf", [P, SB + TB], F32, kind="ExternalInput")
    out = nc.dram_tensor("out", [S, T], BF16, kind="ExternalOutput")

    src_r = src.rearrange("(k p) d -> p k d", p=P)
    tgt_r = tgt.rearrange("(k p) d -> p k d", p=P)
    out_pm = out.rearrange("(m p) n -> p m n", p=P)  # [P, 16, 2048]

    with tile.TileContext(nc) as tc:
        with (
            tc.tile_pool(name="singles", bufs=1) as singles,
            tc.tile_pool(name="inbuf", bufs=1) as inbuf,
            tc.tile_pool(name="sq", bufs=2) as sqp,
            tc.tile_pool(name="scl", bufs=4) as sclp,
            tc.tile_pool(name="pst", bufs=2, space="PSUM") as pst,
            tc.tile_pool(name="psmm", bufs=3, space="PSUM") as psmm,
            tc.tile_pool(name="bandp", bufs=1) as bandp,
            tc.tile_pool(name="outp", bufs=3) as outp,
        ):
            ident = singles.tile([P, P], BF16)
            make_identity(nc, ident)

            mask_sb = singles.tile([P, SB + TB], F32)

            s_nat = inbuf.tile([P, SB, D], BF16)
            t_nat = inbuf.tile([P, TB, D], BF16)
            sT = inbuf.tile([P, S], BF16)  # [D, s tokens] raw (scale in relu)
            tT = inbuf.tile([P, T], BF16)  # [D, t tokens] normalized+masked

            ss_t = singles.tile([P, TB], F32)
            rc_t = singles.tile([P, TB], F32)
            rq_t = singles.tile([P, TB], F32)
            ss_s = singles.tile([P, SB], F32)
            rc_s = singles.tile([P, SB], F32)
            rq_s = singles.tile([P, SB], F32)
            s_scl = singles.tile([P, SB], F32)  # rsqrt * mask, per s block

            # ---- input DMAs: two queues in parallel. sync HWDGE carries the
            # t stream + s03 (FIFO order == drain order); gpsimd SWDGE carries
            # the mask + s tail concurrently.
            nc.sync.dma_start(out=t_nat[:, 0:4, :], in_=tgt_r[:, 0:4, :])
            nc.sync.dma_start(out=s_nat[:, 0:4, :], in_=src_r[:, 0:4, :])
            nc.sync.dma_start(out=t_nat[:, 4:8, :], in_=tgt_r[:, 4:8, :])
            nc.sync.dma_start(out=t_nat[:, 8:16, :], in_=tgt_r[:, 8:16, :])
            nc.gpsimd.dma_start(out=mask_sb, in_=maskf.rearrange("p k -> p k"))
            nc.gpsimd.dma_start(out=s_nat[:, 4:16, :], in_=src_r[:, 4:16, :])

            def norm(x_nat, ss, rc, rq, lo, n, tag):
                """sum-of-squares (gpsimd) -> 1/x (DVE) -> sqrt (ACT)."""
                sq = sqp.tile([P, n, D], BF16, tag="sq", name=f"sq_{tag}{lo}")
                nc.gpsimd.tensor_mul(
                    out=sq, in0=x_nat[:, lo : lo + n, :], in1=x_nat[:, lo : lo + n, :]
                )
                nc.vector.reduce_sum(out=ss[:, lo : lo + n], in_=sq, axis=AX.X)
                nc.vector.reciprocal(out=rc[:, lo : lo + n], in_=ss[:, lo : lo + n])
                nc.scalar.activation(
                    out=rq[:, lo : lo + n], in_=rc[:, lo : lo + n], func=AF.Sqrt
                )

            def xpose_t(lo, n):
                """scale+mask t blocks (gpsimd), PE-transpose, copy to tT."""
                ps = pst.tile([P, 1024], BF16, tag="pst", name=f"xpt{lo}")
                for j in range(n):
                    k = lo + j
                    xs = sclp.tile([P, P], BF16, tag="scl")
                    nc.gpsimd.tensor_scalar(
                        out=xs,
                        in0=t_nat[:, k, :],
                        scalar1=rq_t[:, k : k + 1],
                        scalar2=mask_sb[:, SB + k : SB + k + 1],
                        op0=ALU.mult,
                        op1=ALU.mult,
                    )
                    nc.tensor.transpose(ps[:, j * P : (j + 1) * P], xs, ident)
                half = n * P // 2
                base = lo * P
                nc.scalar.copy(out=tT[:, base : base + half], in_=ps[:, 0:half])
                nc.vector.tensor_copy(
                    out=tT[:, base + half : base + n * P], in_=ps[:, half : n * P]
                )

            def xpose_s(lo, n):
                """PE-transpose raw s blocks (depends only on the s DMA)."""
                ps = pst.tile([P, 1024], BF16, tag="pst", name=f"xps{lo}")
                for j in range(n):
                    k = lo + j
                    nc.tensor.transpose(ps[:, j * P : (j + 1) * P], s_nat[:, k, :], ident)
                half = n * P // 2
                base = lo * P
                nc.scalar.copy(out=sT[:, base : base + half], in_=ps[:, 0:half])
                nc.vector.tensor_copy(
                    out=sT[:, base + half : base + n * P], in_=ps[:, half : n * P]
                )

            def s_mask(lo, n):
                nc.gpsimd.tensor_mul(
                    out=s_scl[:, lo : lo + n],
                    in0=rq_s[:, lo : lo + n],
                    in1=mask_sb[:, lo : lo + n],
                )

            alt = [0]

            def out_op(dst, ps_ap, m):
                """relu(scale * psum) -> bf16 SBUF, alternating ACT/DVE."""
                if alt[0] % 2 == 0:
                    nc.scalar.activation(
                        out=dst, in_=ps_ap, func=AF.Relu, scale=s_scl[:, m : m + 1]
                    )
                else:
                    nc.vector.tensor_scalar(
                        out=dst,
                        in0=ps_ap,
                        scalar1=s_scl[:, m : m + 1],
                        scalar2=0.0,
                        op0=ALU.mult,
                        op1=ALU.max,
                    )
                alt[0] += 1

            # band output tiles for rows 0-3: [P, m-pair, T]
            ob01 = bandp.tile([P, 2, T], BF16, name="ob01")
            ob23 = bandp.tile([P, 2, T], BF16, name="ob23")
            band_obs = [(ob01, 0), (ob23, 2)]

            def band_seg(c0):
                """rows 0-3, columns [c0, c0+512)."""
                for ob, mlo in band_obs:
                    ps = psmm.tile([P, 1024], F32, tag="mm", name=f"b{c0}_{mlo}")
                    for i in range(2):
                        m = mlo + i
                        nc.tensor.matmul(
                            ps[:, i * 512 : (i + 1) * 512],
                            sT[:, m * P : (m + 1) * P],
                            tT[:, c0 : c0 + 512],
                            start=True,
                            stop=True,
                        )
                    for i in range(2):
                        m = mlo + i
                        out_op(
                            ob[:, i, c0 : c0 + 512], ps[:, i * 512 : (i + 1) * 512], m
                        )
                    nc.sync.dma_start(
                        out=out_pm[:, mlo : mlo + 2, c0 : c0 + 512],
                        in_=ob[:, :, c0 : c0 + 512],
                    )

            def band_segwide():
                """rows 0-3, columns [1024, 2048)."""
                for ob, mlo in band_obs:
                    for i in range(2):
                        m = mlo + i
                        ps = psmm.tile([P, 1024], F32, tag="mm", name=f"bD_{m}")
                        nc.tensor.matmul(
                            ps[:, 0:512],
                            sT[:, m * P : (m + 1) * P],
                            tT[:, 1024:1536],
                            start=True,
                            stop=True,
                        )
                        nc.tensor.matmul(
                            ps[:, 512:1024],
                            sT[:, m * P : (m + 1) * P],
                            tT[:, 1536:2048],
                            start=True,
                            stop=True,
                        )
                        out_op(ob[:, i, 1024:2048], ps, m)
                    nc.sync.dma_start(
                        out=out_pm[:, mlo : mlo + 2, 1024:2048],
                        in_=ob[:, :, 1024:2048],
                    )

            def row_pair(mlo):
                """rows mlo, mlo+1 row-major; one 1 MB pair DMA."""
                obp = outp.tile([P, 2, T], BF16, tag="obp", name=f"obp{mlo}")
                for i in range(2):
                    m = mlo + i
                    for h in range(2):
                        c = h * 1024
                        ps = psmm.tile([P, 1024], F32, tag="mm", name=f"r{m}_{h}")
                        nc.tensor.matmul(
                            ps[:, 0:512],
                            sT[:, m * P : (m + 1) * P],
                            tT[:, c : c + 512],
                            start=True,
                            stop=True,
                        )
                        nc.tensor.matmul(
                            ps[:, 512:1024],
                            sT[:, m * P : (m + 1) * P],
                            tT[:, c + 512 : c + 1024],
                            start=True,
                            stop=True,
                        )
                        out_op(obp[:, i, c : c + 1024], ps, m)
                nc.sync.dma_start(out=out_pm[:, mlo : mlo + 2, :], in_=obp)

            # ---- emission order == per-engine FIFO order. Per engine, ops
            # are emitted in the order their inputs become ready so no engine
            # head-of-line-blocks on a semaphore while later work is ready.
            norm(t_nat, ss_t, rc_t, rq_t, 0, 4, "t")  # t0-3
            norm(s_nat, ss_s, rc_s, rq_s, 0, 4, "s")  # s0-3 (for s_scl)
            xpose_s(0, 4)  # PE head: only needs the s03 DMA
            xpose_t(0, 4)
            s_mask(0, 4)
            norm(t_nat, ss_t, rc_t, rq_t, 4, 4, "t")  # t4-7
            band_seg(0)
            xpose_t(4, 4)
            norm(t_nat, ss_t, rc_t, rq_t, 8, 4, "t")
            band_seg(512)
            xpose_t(8, 4)
            norm(t_nat, ss_t, rc_t, rq_t, 12, 4, "t")
            xpose_t(12, 4)
            norm(s_nat, ss_s, rc_s, rq_s, 4, 4, "s")
            s_mask(4, 4)
            band_segwide()
            xpose_s(4, 4)
            row_pair(4)
            norm(s_nat, ss_s, rc_s, rq_s, 8, 4, "s")
            s_mask(8, 4)
            row_pair(6)
            xpose_s(8, 4)
            row_pair(8)
            norm(s_nat, ss_s, rc_s, rq_s, 12, 4, "s")
            s_mask(12, 4)
            row_pair(10)
            xpose_s(12, 4)
            row_pair(12)
            row_pair(14)

    nc.compile()
    return nc


_NC_CACHE = None


def _get_nc():
    global _NC_CACHE
    if _NC_CACHE is None:
        _NC_CACHE = build_nc()
    return _NC_CACHE


def kernel(source, target, mask_src, mask_tar, **run_kwargs):
    source = np.asarray(source, dtype=np.float32)
    target = np.asarray(target, dtype=np.float32)
    mask_src = np.asarray(mask_src)
    mask_tar = np.asarray(mask_tar)
    B = source.shape[0]

    in_maps = []
    for b in range(B):
        msf = mask_src[b].astype(np.float32).reshape(SB, P).T
        mtf = mask_tar[b].astype(np.float32).reshape(TB, P).T
        mk = np.ascontiguousarray(np.concatenate([msf, mtf], axis=1))
        in_maps.append(
            {
                "src": np.ascontiguousarray(source[b].astype(BF)),
                "tgt": np.ascontiguousarray(target[b].astype(BF)),
                "maskf": mk,
            }
        )

    nc = _get_nc()
    res = run_bass_kernel_spmd(nc, in_maps, core_ids=list(range(B)), **run_kwargs)
    out = np.stack(
        [np.asarray(r["out"]).astype(np.float32) for r in res.results], axis=0
    )
    if run_kwargs.get("trace"):
        kernel.last_results = res
    return out


# revision 6
# speedup vs baseline: 1.3233x; 1.0072x over previous
"""Trainium2 Bass kernel for nn_Jointer: per-sample masked cosine-similarity.

out[b] = relu(l2norm(source[b]) @ l2norm(target[b]).T) * (mask_src[b] outer mask_tar[b])

Sharding: data-parallel over batch B=8 -> one sample per NeuronCore.

Strategy (memory-bound problem; rel-err budget 2e-2 permits bf16 I/O):
- Host casts source/target to bf16 (halves input DMA bytes); kernel writes a
  bf16 output that the host upcasts to f32 (halves the dominant 16 MB output
  stream). Norm statistics and matmul accumulation stay fp32; measured end-to-
  end rel err ~3.3e-3.
- Engine roles: GpSimd does the SBUF-side elementwise prep (squares, reduces,
  scale*mask) so the DVE/ACT FIFOs stay clear for the PSUM-drain relu ops;
  DVE does reciprocals + half the relu ops + half the transpose copies; ACT
  does sqrt + the other halves. PE does transposes + matmuls only.
- t operands are scaled by rsqrt(ss)*mask BEFORE the PE transpose; s operands
  are transposed RAW and their rsqrt(ss)*mask scale is fused into the
  PSUM->SBUF relu pass (per-row scalar), so the s-transposes depend only on
  the s input DMA.
- Inputs load via two queues in parallel (sync HWDGE + gpsimd SWDGE); outputs
  stream on the sync queue as a column-major "band" over rows 0-3 (each band
  segment needs only the t-blocks transposed so far), then rows 4-15 go
  row-major with 1 MB row-pair DMAs.
- PSUM->SBUF relu ops work on [128,1024] two-bank tiles to amortize the ACT
  engine's fixed overhead; ops alternate ACT/DVE. Transpose PSUM tiles are
  padded to a full bank so PE writes never share a bank with ACT/DVE reads.
"""

import numpy as np
import ml_dtypes

import concourse.bass as bass
from concourse import bacc
import concourse.mybir as mybir
import concourse.tile as tile
from concourse.bass_utils import run_bass_kernel_spmd
from concourse.masks import make_identity

F32 = mybir.dt.float32
BF16 = mybir.dt.bfloat16
AF = mybir.ActivationFunctionType
ALU = mybir.AluOpType
AX = mybir.AxisListType

S = 2048  # source tokens per sample
T = 2048  # target tokens per sample
D = 128  # feature dim (= contraction dim = partitions)
P = 128  # partitions
SB = S // P  # 16 source token blocks
TB = T // P  # 16 target token blocks

BF = ml_dtypes.bfloat16


def build_nc() -> bass.Bass:
    nc = bacc.Bacc(trn_type="TRN2")

    src = nc.dram_tensor("src", [S, D], BF16, kind="ExternalInput")
    tgt = nc.dram_tensor("tgt", [T, D], BF16, kind="ExternalInput")
    # maskf[p, k]: k in [0,16) source-block masks, k in [16,32) target-block
    # masks; value for token 128*k + p.
    maskf = nc.dram_tensor("maskf", [P, SB + TB], F32, kind="ExternalInput")
    out = nc.dram_tensor("out", [S, T], BF16, kind="ExternalOutput")

    src_r = src.rearrange("(k p) d -> p k d", p=P)
    tgt_r = tgt.rearrange("(k p) d -> p k d", p=P)
    out_pm = out.rearrange("(m p) n -> p m n", p=P)  # [P, 16, 2048]

    with tile.TileContext(nc) as tc:
        with (
            tc.tile_pool(name="singles", bufs=1) as singles,
            tc.tile_pool(name="inbuf", bufs=1) as inbuf,
            tc.tile_pool(name="sq", bufs=2) as sqp,
            tc.tile_pool(name="scl", bufs=4) as sclp,
            tc.tile_pool(name="pst", bufs=2, space="PSUM") as pst,
            tc.tile_pool(name="psmm", bufs=3, space="PSUM") as psmm,
            tc.tile_pool(name="bandp", bufs=1) as bandp,
            tc.tile_pool(name="outp", bufs=3) as outp,
        ):
            ident = singles.tile([P, P], BF16)
            make_identity(nc, ident)

            # First ACT-stream instruction must be a Sqrt so the compiler
            # loads the sqrt table set (which also contains relu/copy) once;
            # otherwise a Copy-first stream loads a different set and the
            # switch lands on the ramp critical path.
            sqrt_warm = singles.tile([P, 1], F32)
            nc.scalar.activation(out=sqrt_warm, in_=ident[:, 0:1], func=AF.Sqrt)

            mask_sb = singles.tile([P, SB + TB], F32)

            s_nat = inbuf.tile([P, SB, D], BF16)
            t_nat = inbuf.tile([P, TB, D], BF16)
            sT = inbuf.tile([P, S], BF16)  # [D, s tokens] raw (scale in relu)
            tT = inbuf.tile([P, T], BF16)  # [D, t tokens] normalized+masked

            ss_t = singles.tile([P, TB], F32)
            rc_t = singles.tile([P, TB], F32)
            rq_t = singles.tile([P, TB], F32)
            ss_s = singles.tile([P, SB], F32)
            rc_s = singles.tile([P, SB], F32)
            rq_s = singles.tile([P, SB], F32)
            s_scl = singles.tile([P, SB], F32)  # rsqrt * mask, per s block

            # ---- input DMAs: two queues in parallel. sync HWDGE carries the
            # t stream + s03 (FIFO order == drain order); gpsimd SWDGE carries
            # the mask + s tail concurrently.
            nc.sync.dma_start(out=t_nat[:, 0:4, :], in_=tgt_r[:, 0:4, :])
            nc.sync.dma_start(out=s_nat[:, 0:4, :], in_=src_r[:, 0:4, :])
            nc.sync.dma_start(out=t_nat[:, 4:8, :], in_=tgt_r[:, 4:8, :])
            nc.sync.dma_start(out=t_nat[:, 8:16, :], in_=tgt_r[:, 8:16, :])
            nc.gpsimd.dma_start(out=mask_sb, in_=maskf.rearrange("p k -> p k"))
            nc.gpsimd.dma_start(out=s_nat[:, 4:16, :], in_=src_r[:, 4:16, :])

            def norm(x_nat, ss, rc, rq, lo, n, tag):
                """sum-of-squares (gpsimd) -> 1/x (DVE) -> sqrt (ACT)."""
                sq = sqp.tile([P, n, D], BF16, tag="sq", name=f"sq_{tag}{lo}")
                nc.vector.tensor_mul(
                    out=sq, in0=x_nat[:, lo : lo + n, :], in1=x_nat[:, lo : lo + n, :]
                )
                nc.vector.reduce_sum(out=ss[:, lo : lo + n], in_=sq, axis=AX.X)
                nc.vector.reciprocal(out=rc[:, lo : lo + n], in_=ss[:, lo : lo + n])
                nc.scalar.activation(
                    out=rq[:, lo : lo + n], in_=rc[:, lo : lo + n], func=AF.Sqrt
                )

            def xpose_t(lo, n):
                """scale+mask t blocks (gpsimd), PE-transpose, copy to tT."""
                ps = pst.tile([P, 1024], BF16, tag="pst", name=f"xpt{lo}")
                for j in range(n):
                    k = lo + j
                    xs = sclp.tile([P, P], BF16, tag="scl")
                    nc.gpsimd.tensor_scalar(
                        out=xs,
                        in0=t_nat[:, k, :],
                        scalar1=rq_t[:, k : k + 1],
                        scalar2=mask_sb[:, SB + k : SB + k + 1],
                        op0=ALU.mult,
                        op1=ALU.mult,
                    )
                    nc.tensor.transpose(ps[:, j * P : (j + 1) * P], xs, ident)
                base = lo * P
                nc.scalar.copy(out=tT[:, base : base + n * P], in_=ps[:, 0 : n * P])

            def xpose_s(lo, n):
                """PE-transpose raw s blocks (depends only on the s DMA)."""
                ps = pst.tile([P, 1024], BF16, tag="pst", name=f"xps{lo}")
                for j in range(n):
                    k = lo + j
                    nc.tensor.transpose(ps[:, j * P : (j + 1) * P], s_nat[:, k, :], ident)
                base = lo * P
                nc.scalar.copy(out=sT[:, base : base + n * P], in_=ps[:, 0 : n * P])

            def s_mask(lo, n):
                nc.gpsimd.tensor_mul(
                    out=s_scl[:, lo : lo + n],
                    in0=rq_s[:, lo : lo + n],
                    in1=mask_sb[:, lo : lo + n],
                )

            alt = [0]

            def out_op(dst, ps_ap, m):
                """relu(scale * psum) -> bf16 SBUF, alternating ACT/DVE."""
                if alt[0] % 3 != 2:
                    nc.scalar.activation(
                        out=dst, in_=ps_ap, func=AF.Relu, scale=s_scl[:, m : m + 1]
                    )
                else:
                    nc.vector.tensor_scalar(
                        out=dst,
                        in0=ps_ap,
                        scalar1=s_scl[:, m : m + 1],
                        scalar2=0.0,
                        op0=ALU.mult,
                        op1=ALU.max,
                    )
                alt[0] += 1

            # band output tiles for rows 0-3: [P, m-pair, T]
            ob01 = bandp.tile([P, 2, T], BF16, name="ob01")
            ob23 = bandp.tile([P, 2, T], BF16, name="ob23")
            band_obs = [(ob01, 0), (ob23, 2)]

            def band_seg(c0):
                """rows 0-3, columns [c0, c0+512)."""
                for ob, mlo in band_obs:
                    ps = psmm.tile([P, 1024], F32, tag="mm", name=f"b{c0}_{mlo}")
                    for i in range(2):
                        m = mlo + i
                        nc.tensor.matmul(
                            ps[:, i * 512 : (i + 1) * 512],
                            sT[:, m * P : (m + 1) * P],
                            tT[:, c0 : c0 + 512],
                            start=True,
                            stop=True,
                        )
                    for i in range(2):
                        m = mlo + i
                        out_op(
                            ob[:, i, c0 : c0 + 512], ps[:, i * 512 : (i + 1) * 512], m
                        )
                    nc.sync.dma_start(
                        out=out_pm[:, mlo : mlo + 2, c0 : c0 + 512],
                        in_=ob[:, :, c0 : c0 + 512],
                    )

            def band_segwide():
                """rows 0-3, columns [1024, 2048)."""
                for ob, mlo in band_obs:
                    for i in range(2):
                        m = mlo + i
                        ps = psmm.tile([P, 1024], F32, tag="mm", name=f"bD_{m}")
                        nc.tensor.matmul(
                            ps[:, 0:512],
                            sT[:, m * P : (m + 1) * P],
                            tT[:, 1024:1536],
                            start=True,
                            stop=True,
                        )
                        nc.tensor.matmul(
                            ps[:, 512:1024],
                            sT[:, m * P : (m + 1) * P],
                            tT[:, 1536:2048],
                            start=True,
                            stop=True,
                        )
                        out_op(ob[:, i, 1024:2048], ps, m)
                    nc.sync.dma_start(
                        out=out_pm[:, mlo : mlo + 2, 1024:2048],
                        in_=ob[:, :, 1024:2048],
                    )

            def row_single(m):
                obs = outp.tile([P, T], BF16, tag="obs", name=f"obs{m}")
                for h in range(2):
                    c = h * 1024
                    ps = psmm.tile([P, 1024], F32, tag="mm", name=f"r{m}_{h}")
                    nc.tensor.matmul(
                        ps[:, 0:512],
                        sT[:, m * P : (m + 1) * P],
                        tT[:, c : c + 512],
                        start=True,
                        stop=True,
                    )
                    nc.tensor.matmul(
                        ps[:, 512:1024],
                        sT[:, m * P : (m + 1) * P],
                        tT[:, c + 512 : c + 1024],
                        start=True,
                        stop=True,
                    )
                    out_op(obs[:, c : c + 1024], ps, m)
                    nc.sync.dma_start(
                        out=out_pm[:, m, c : c + 1024], in_=obs[:, c : c + 1024]
                    )

            def row_pair(mlo):
                """rows mlo, mlo+1 row-major; one 1 MB pair DMA."""
                obp = outp.tile([P, 2, T], BF16, tag="obp", name=f"obp{mlo}")
                for i in range(2):
                    m = mlo + i
                    for h in range(2):
                        c = h * 1024
                        ps = psmm.tile([P, 1024], F32, tag="mm", name=f"r{m}_{h}")
                        nc.tensor.matmul(
                            ps[:, 0:512],
                            sT[:, m * P : (m + 1) * P],
                            tT[:, c : c + 512],
                            start=True,
                            stop=True,
                        )
                        nc.tensor.matmul(
                            ps[:, 512:1024],
                            sT[:, m * P : (m + 1) * P],
                            tT[:, c + 512 : c + 1024],
                            start=True,
                            stop=True,
                        )
                        out_op(obp[:, i, c : c + 1024], ps, m)
                nc.sync.dma_start(out=out_pm[:, mlo : mlo + 2, :], in_=obp)

            # ---- emission order == per-engine FIFO order. Per engine, ops
            # are emitted in the order their inputs become ready so no engine
            # head-of-line-blocks on a semaphore while later work is ready.
            norm(t_nat, ss_t, rc_t, rq_t, 0, 4, "t")  # t0-3
            norm(s_nat, ss_s, rc_s, rq_s, 0, 4, "s")  # s0-3 (for s_scl)
            xpose_s(0, 4)  # PE head: only needs the s03 DMA
            xpose_t(0, 4)
            s_mask(0, 4)
            norm(t_nat, ss_t, rc_t, rq_t, 4, 4, "t")  # t4-7
            band_seg(0)
            xpose_t(4, 4)
            norm(t_nat, ss_t, rc_t, rq_t, 8, 4, "t")
            band_seg(512)
            xpose_t(8, 4)
            norm(t_nat, ss_t, rc_t, rq_t, 12, 4, "t")
            xpose_t(12, 4)
            norm(s_nat, ss_s, rc_s, rq_s, 4, 4, "s")
            s_mask(4, 4)
            band_segwide()
            xpose_s(4, 4)
            row_pair(4)
            norm(s_nat, ss_s, rc_s, rq_s, 8, 4, "s")
            s_mask(8, 4)
            row_pair(6)
            xpose_s(8, 4)
            row_pair(8)
            norm(s_nat, ss_s, rc_s, rq_s, 12, 4, "s")
            s_mask(12, 4)
            row_pair(10)
            xpose_s(12, 4)
            row_pair(12)
            row_single(14)
            row_single(15)

    nc.compile()
    return nc


_NC_CACHE = None


def _get_nc():
    global _NC_CACHE
    if _NC_CACHE is None:
        _NC_CACHE = build_nc()
    return _NC_CACHE


def kernel(source, target, mask_src, mask_tar, **run_kwargs):
    source = np.asarray(source, dtype=np.float32)
    target = np.asarray(target, dtype=np.float32)
    mask_src = np.asarray(mask_src)
    mask_tar = np.asarray(mask_tar)
    B = source.shape[0]

    in_maps = []
    for b in range(B):
        msf = mask_src[b].astype(np.float32).reshape(SB, P).T
        mtf = mask_tar[b].astype(np.float32).reshape(TB, P).T
        mk = np.ascontiguousarray(np.concatenate([msf, mtf], axis=1))
        in_maps.append(
            {
                "src": np.ascontiguousarray(source[b].astype(BF)),
                "tgt": np.ascontiguousarray(target[b].astype(BF)),
                "maskf": mk,
            }
        )

    nc = _get_nc()
    res = run_bass_kernel_spmd(nc, in_maps, core_ids=list(range(B)), **run_kwargs)
    out = np.stack(
        [np.asarray(r["out"]).astype(np.float32) for r in res.results], axis=0
    )
    if run_kwargs.get("trace"):
        kernel.last_results = res
    return out
